# revision 27
# baseline (speedup 1.0000x reference)
"""Causal self-attention (B=2, T=2048, C=1024, H=16) on 8 trn2 NeuronCores.

Sharding: core c -> batch b=c//4 (data parallel) x head-group g=c%4
(tensor parallel, 4 heads each). Each core computes qkv projections for
its 4 heads, causal flash-style attention, and a partial output
projection (its heads' rows of W_proj); the host sums the 4 partials
per batch and adds b_proj.

Device layout avoids all on-chip transposes:
  - q,k are produced transposed ([head_dim*heads, T], dims on partitions)
    straight out of the qkv matmul (W as stationary, xT as moving).
  - v is produced in natural [T, head_dim] layout (xT as stationary),
    augmented with a ones column so P@v_aug also yields the softmax
    denominator.
  - scores are computed as S^T = k @ q^T in [s, t] layout, exp'ed with
    no max subtraction (scores are O(1); fully-masked blocks are
    skipped, diagonal blocks multiplied by a 0/1 triangular mask, which
    matches exp(-10000+...) == 0.0 in fp32 exactly).
  - y^T = v_aug^T @ P accumulates in PSUM; the denominator row is
    reciprocal'ed and broadcast across partitions via a tiny selector
    matmul, then multiplied in.
  - the output projection consumes y^T directly as the moving operand
    (W_proj slice stationary), producing the partial output transposed.

All matmul operands are float32r (1 cycle/row at N>=512 vs 4 for fp32;
measured dot-product rel-err ~1.5e-4 at K=128).
"""

import numpy as np

import concourse.bass as bass
import concourse.tile as tile
from concourse import bacc, mybir
from concourse.bass_utils import run_bass_kernel_spmd

F32 = mybir.dt.float32
F32R = mybir.dt.float32r
EXP = mybir.ActivationFunctionType.Exp
IDENT = mybir.ActivationFunctionType.Identity

B, T, C, H = 2, 2048, 1024, 16
D = C // H                    # 64
N_CORES = 8
HG = 4                        # heads per core
TCH = 512                     # t-chunk (moving free dim)
NJ = T // TCH                 # 4 t-chunks
NS = T // 128                 # 16 s-tiles
KC = C // 128                 # 8 contraction chunks


def _emit(nc, tc, io):
    import contextlib
    ctx = contextlib.ExitStack()
    with ctx:
        const = ctx.enter_context(tc.tile_pool(name="const", bufs=1))
        xp = ctx.enter_context(tc.tile_pool(name="xp", bufs=8))
        qkp = ctx.enter_context(tc.tile_pool(name="qkp", bufs=1))
        vp = ctx.enter_context(tc.tile_pool(name="vp", bufs=1))
        pp = ctx.enter_context(tc.tile_pool(name="pp", bufs=4))
        yp = ctx.enter_context(tc.tile_pool(name="yp", bufs=1))
        op = ctx.enter_context(tc.tile_pool(name="op", bufs=3))
        ps = ctx.enter_context(tc.tile_pool(name="ps", bufs=6, space="PSUM"))

        # ---- constants / weights ----
        wqk_t = []
        wv_t = []
        for c in range(KC):
            w1 = const.tile([128, 512], F32R, name=f"wqk{c}")
            nc.sync.dma_start(w1[:], io["wqk"][128 * c:128 * (c + 1), :])
            wqk_t.append(w1)
            w2 = const.tile([128, 256], F32R, name=f"wv{c}")
            nc.sync.dma_start(w2[:], io["wv"][128 * c:128 * (c + 1), :])
            wv_t.append(w2)
        wp_t = []
        for p in range(2):
            w3 = const.tile([128, 1024], F32R, name=f"wp{p}")
            nc.sync.dma_start(w3[:], io["wp"][128 * p:128 * (p + 1), :])
            wp_t.append(w3)
        bqk_t = const.tile([128, 4], F32, name="bqk")
        nc.sync.dma_start(bqk_t[:], io["bqk"][:])
        bv_t = const.tile([128, 256], F32, name="bv")
        nc.sync.dma_start(bv_t[:], io["bv"][:])
        am_t = const.tile([128, NS], F32, name="am")
        nc.sync.dma_start(am_t[:], io["amask"][:])
        cm_t = const.tile([128, 1024], F32R, name="cm")
        nc.sync.dma_start(cm_t[:], io["cmask"][:])
        ones_t = const.tile([128, 64], F32R, name="ones")
        nc.sync.dma_start(ones_t[:], io["ones"][:])

        # ---- qkv outputs ----
        # qk_tiles m-chunks: 0: qT heads {0,1}; 1: qT heads {2,3};
        #                    2: kT heads {0,1}; 3: kT heads {2,3}
        qk_tiles = [qkp.tile([128, T], F32R, name=f"qk{m}") for m in range(4)]
        v_big = [vp.tile([128, 65 * NS], F32R, name=f"vb{h}") for h in range(HG)]
        for h in range(HG):
            # ones column at position 64 of every 65-wide block (softmax
            # denominator accumulator); memset can't write f32r, so copy
            # from the ones tile through a strided AP.
            onescol = v_big[h][:].rearrange("p (s c) -> p s c", c=65)[:, :, 64]
            nc.vector.tensor_copy(onescol, ones_t[:, 0:NS])

        # ---- qkv projection, in two t-halves to bound xT residency ----
        for half in range(2):
            xt = []
            for c in range(KC):
                x1 = xp.tile([128, 1024], F32R, name="xt", tag="xt")
                nc.sync.dma_start(
                    x1[:], io["xT"][128 * c:128 * (c + 1),
                                    1024 * half:1024 * (half + 1)])
                xt.append(x1)
            # qT / kT: weights stationary, xT moving -> transposed outputs
            for m in range(4):
                for tj in range(2):
                    j = 2 * half + tj
                    pq = ps.tile([128, 512], F32, name="pq", tag="ps")
                    for c in range(KC):
                        nc.tensor.matmul(
                            pq[:], wqk_t[c][:, 128 * m:128 * (m + 1)],
                            xt[c][:, 512 * tj:512 * (tj + 1)],
                            start=(c == 0), stop=(c == KC - 1))
                    nc.scalar.activation(
                        qk_tiles[m][:, TCH * j:TCH * (j + 1)], pq[:], IDENT,
                        bias=bqk_t[:, m:m + 1], scale=1.0)
            # v: xT stationary, Wv moving -> natural [s, d] layout
            for si in range(NS // 2):
                s = NS // 2 * half + si
                pv = ps.tile([128, 256], F32, name="pv", tag="ps")
                for c in range(KC):
                    nc.tensor.matmul(
                        pv[:], xt[c][:, 128 * si:128 * (si + 1)], wv_t[c][:],
                        start=(c == 0), stop=(c == KC - 1))
                for h in range(HG):
                    nc.vector.tensor_add(
                        v_big[h][:, 65 * s:65 * s + 64],
                        pv[:, 64 * h:64 * (h + 1)], bv_t[:, 64 * h:64 * (h + 1)])

        # ---- attention ----
        yT = [yp.tile([128, T], F32R, name=f"yT{p}") for p in range(2)]
        # denominator rows: head h -> tile h//2, partition 32*(h%2)
        # (engine APs may only start at partition 0/32/64; 96 is illegal)
        l_t = [yp.tile([64, T], F32, name=f"l{p}") for p in range(2)]
        rl_t = [yp.tile([64, T], F32R, name=f"rl{p}") for p in range(2)]

        # chunk-major so the output projection for chunk j can start as
        # soon as all 4 heads finish chunk j (no serial proj tail).
        for j in range(NJ):
            cols = slice(TCH * j, TCH * (j + 1))
            for h in range(HG):
                pr, hh = divmod(h, 2)
                qt, kt = qk_tiles[pr], qk_tiles[2 + pr]
                rows = slice(64 * hh, 64 * (hh + 1))
                py = ps.tile([128, 512], F32, name="py", tag="ps")
                ns = 4 * (j + 1)
                for i in range(ns):
                    r = i - 4 * j
                    # diagonal blocks r=1,2: the first 128*r t-columns are
                    # entirely below the causal frontier -> skip them.
                    # (r=3 keeps N=512: N=128 would hit the f32r 4x
                    # slow path below N=256.)
                    off = 128 * r if r in (1, 2) else 0
                    sub = slice(off, 512)
                    tsub = slice(TCH * j + off, TCH * (j + 1))
                    pscr = ps.tile([128, 512], F32, name="pscr", tag="ps")
                    nc.tensor.matmul(
                        pscr[:, sub], kt[rows, 128 * i:128 * (i + 1)],
                        qt[rows, tsub], start=True, stop=True)
                    pt = pp.tile([128, 512], F32R, name="pt", tag="pt")
                    nc.scalar.activation(
                        pt[:, sub], pscr[:, sub], EXP,
                        bias=am_t[:, i:i + 1], scale=1.0 / np.sqrt(D))
                    if r >= 0:  # 0/1 triangular mask on the boundary
                        if r == 3:
                            mask = cm_t[:, 512:1024]
                        else:
                            mask = cm_t[:, 0:512 - off]
                        nc.vector.tensor_mul(pt[:, sub], pt[:, sub], mask)
                    nc.tensor.matmul(
                        py[0:65, sub], v_big[h][:, 65 * i:65 * (i + 1)],
                        pt[:, sub], start=(i == 0), stop=(i == ns - 1))
                # drain y rows + denominator row, reciprocal, then
                # broadcast the recip row across 64 partitions via a
                # K=1 ones-row matmul and normalize in place.
                lr = 32 * hh
                nc.vector.tensor_copy(yT[pr][rows, cols], py[0:64, :])
                nc.vector.tensor_copy(l_t[pr][lr:lr + 1, cols],
                                      py[64:65, :])
                nc.vector.reciprocal(rl_t[pr][lr:lr + 1, cols],
                                     l_t[pr][lr:lr + 1, cols])
                pb = ps.tile([128, 512], F32, name="pb", tag="ps")
                nc.tensor.matmul(
                    pb[0:64, :], ones_t[lr:lr + 1, :],
                    rl_t[pr][lr:lr + 1, cols],
                    start=True, stop=True)
                nc.vector.tensor_mul(
                    yT[pr][rows, cols], yT[pr][rows, cols], pb[0:64, :])

            # ---- output projection for chunk j (partial; host sums) ----
            for m in range(8):
                po = ps.tile([128, 512], F32, name="po", tag="ps")
                for pr in range(2):
                    nc.tensor.matmul(
                        po[:], wp_t[pr][:, 128 * m:128 * (m + 1)],
                        yT[pr][:, cols],
                        start=(pr == 0), stop=(pr == 1))
                ot = op.tile([128, 512], F32, name="ot", tag="ot")
                nc.vector.tensor_copy(ot[:], po[:])
                nc.sync.dma_start(
                    io["out"][128 * m:128 * (m + 1), cols], ot[:])


def _build():
    nc = bacc.Bacc("TRN2", target_bir_lowering=False, debug=False)
    io = {
        "xT": nc.dram_tensor("xT", [C, T], F32R, kind="ExternalInput").ap(),
        "wqk": nc.dram_tensor("wqk", [C, 512], F32R, kind="ExternalInput").ap(),
        "wv": nc.dram_tensor("wv", [C, 256], F32R, kind="ExternalInput").ap(),
        "wp": nc.dram_tensor("wp", [256, C], F32R, kind="ExternalInput").ap(),
        "bqk": nc.dram_tensor("bqk", [128, 4], F32, kind="ExternalInput").ap(),
        "bv": nc.dram_tensor("bv", [128, 256], F32, kind="ExternalInput").ap(),
        "amask": nc.dram_tensor("amask", [128, NS], F32, kind="ExternalInput").ap(),
        "cmask": nc.dram_tensor("cmask", [128, 1024], F32R, kind="ExternalInput").ap(),
        "ones": nc.dram_tensor("ones", [128, 64], F32R, kind="ExternalInput").ap(),
        "out": nc.dram_tensor("out", [C, T], F32, kind="ExternalOutput").ap(),
    }
    with nc.allow_low_precision("f32r matmul operand staging"):
        with tile.TileContext(nc) as tc:
            _emit(nc, tc, io)
    nc.compile()
    return nc


_NC_CACHE = {}


def _get_nc():
    if "nc" not in _NC_CACHE:
        _NC_CACHE["nc"] = _build()
    return _NC_CACHE["nc"]


def _host_inputs(x, attention_mask, W_attn, b_attn, W_proj):
    """Per-core input dicts implementing the batch x head-group sharding."""
    # causal 0/1 masks: cols 0:512 = (f >= p) for the r=0/1/2 diagonal
    # blocks (truncated per block), cols 512:1024 = (f >= 384 + p) for r=3
    p = np.arange(128)[:, None]
    f = np.arange(512)[None, :]
    cm = np.concatenate(
        [(f >= p).astype(np.float32),
         (f >= 384 + p).astype(np.float32)], axis=1)
    ones = np.ones((128, 64), np.float32)
    in_maps = []
    for c in range(N_CORES):
        b, g = divmod(c, HG)
        q0 = 256 * g
        wqk = np.ascontiguousarray(np.concatenate(
            [W_attn[:, q0:q0 + 256], W_attn[:, C + q0:C + q0 + 256]], axis=1))
        wv = np.ascontiguousarray(W_attn[:, 2 * C + q0:2 * C + q0 + 256])
        wp = np.ascontiguousarray(W_proj[q0:q0 + 256, :])
        bqk = np.stack(
            [b_attn[q0:q0 + 128], b_attn[q0 + 128:q0 + 256],
             b_attn[C + q0:C + q0 + 128], b_attn[C + q0 + 128:C + q0 + 256]],
            axis=1).astype(np.float32)
        bv = np.broadcast_to(
            b_attn[2 * C + q0:2 * C + q0 + 256], (128, 256)).astype(np.float32)
        am = np.ascontiguousarray(
            attention_mask[b, 0, 0].reshape(NS, 128).T.astype(np.float32))
        xT = np.ascontiguousarray(x[b].T)
        in_maps.append(dict(xT=xT, wqk=wqk, wv=wv, wp=wp, bqk=bqk, bv=bv,
                            amask=am, cmask=cm, ones=ones))
    return in_maps


def _assemble(results, b_proj):
    out = np.empty((B, T, C), np.float32)
    for b in range(B):
        acc = np.zeros((C, T), np.float64)
        for g in range(HG):
            acc += results[HG * b + g]["out"].astype(np.float64)
        out[b] = acc.T + b_proj[None, :]
    return out


def kernel(x, attention_mask, W_attn, b_attn, W_proj, b_proj):
    x = np.asarray(x, np.float32)
    attention_mask = np.asarray(attention_mask, np.float32)
    W_attn = np.asarray(W_attn, np.float32)
    b_attn = np.asarray(b_attn, np.float32)
    W_proj = np.asarray(W_proj, np.float32)
    b_proj = np.asarray(b_proj, np.float32)

    nc = _get_nc()
    in_maps = _host_inputs(x, attention_mask, W_attn, b_attn, W_proj)
    res = run_bass_kernel_spmd(nc, in_maps, list(range(N_CORES)))
    return _assemble(res.results, b_proj)


# revision 30
# speedup vs baseline: 1.4705x; 1.4705x over previous
"""Causal self-attention (B=2, T=2048, C=1024, H=16) on 8 trn2 NeuronCores.

Sharding: core c -> batch b=c//4 (data parallel) x head-group g=c%4
(tensor parallel, 4 heads each). Each core computes qkv projections for
its 4 heads, causal flash-style attention, and a partial output
projection (its heads' rows of W_proj); the host sums the 4 partials
per batch and adds b_proj.

Device layout avoids all on-chip transposes:
  - q,k are produced transposed ([head_dim*heads, T], dims on partitions)
    straight out of the qkv matmul (W as stationary, xT as moving).
  - v is produced in natural [T, head_dim] layout (xT as stationary),
    augmented with a ones column so P@v_aug also yields the softmax
    denominator.
  - scores are computed as S^T = k @ q^T in [s, t] layout, exp'ed with
    no max subtraction (scores are O(1); fully-masked blocks are
    skipped, diagonal blocks multiplied by a 0/1 triangular mask, which
    matches exp(-10000+...) == 0.0 in fp32 exactly).
  - y^T = v_aug^T @ P accumulates in PSUM; the denominator row is
    reciprocal'ed and broadcast across partitions via a tiny selector
    matmul, then multiplied in.
  - the output projection consumes y^T directly as the moving operand
    (W_proj slice stationary), producing the partial output transposed.

All matmul operands are float32r (1 cycle/row at N>=512 vs 4 for fp32;
measured dot-product rel-err ~1.5e-4 at K=128).
"""

import numpy as np

import concourse.bass as bass
import concourse.tile as tile
from concourse import bacc, mybir
from concourse.bass_utils import run_bass_kernel_spmd

F32 = mybir.dt.float32
F32R = mybir.dt.float32r
EXP = mybir.ActivationFunctionType.Exp
IDENT = mybir.ActivationFunctionType.Identity

B, T, C, H = 2, 2048, 1024, 16
D = C // H                    # 64
N_CORES = 8
HG = 4                        # heads per core
TCH = 512                     # t-chunk (moving free dim)
NJ = T // TCH                 # 4 t-chunks
NS = T // 128                 # 16 s-tiles
KC = C // 128                 # 8 contraction chunks


def _emit(nc, tc, io, reps=1):
    import contextlib
    ctx = contextlib.ExitStack()
    with ctx:
        if reps > 1:
            # timing builds only: repeat the whole body on-device so the
            # per-iteration time can be measured free of host dispatch
            # noise ((t(reps) - t(1)) / (reps - 1)).
            hints = (mybir.EngineType.PE, mybir.EngineType.SP,
                     mybir.EngineType.Activation, mybir.EngineType.DVE)
            ctx.enter_context(tc.For_i(0, reps, 1, hint_engines=hints))
        const = ctx.enter_context(tc.tile_pool(name="const", bufs=1))
        xp = ctx.enter_context(tc.tile_pool(name="xp", bufs=8))
        qkp = ctx.enter_context(tc.tile_pool(name="qkp", bufs=1))
        vp = ctx.enter_context(tc.tile_pool(name="vp", bufs=1))
        pp = ctx.enter_context(tc.tile_pool(name="pp", bufs=4))
        yp = ctx.enter_context(tc.tile_pool(name="yp", bufs=1))
        op = ctx.enter_context(tc.tile_pool(name="op", bufs=3))
        ps = ctx.enter_context(tc.tile_pool(name="ps", bufs=6, space="PSUM"))

        # ---- constants / weights ----
        wqk_t = []
        wv_t = []
        for c in range(KC):
            w1 = const.tile([128, 512], F32R, name=f"wqk{c}")
            nc.sync.dma_start(w1[:], io["wqk"][128 * c:128 * (c + 1), :])
            wqk_t.append(w1)
            w2 = const.tile([128, 256], F32R, name=f"wv{c}")
            nc.sync.dma_start(w2[:], io["wv"][128 * c:128 * (c + 1), :])
            wv_t.append(w2)
        wp_t = []
        for p in range(2):
            w3 = const.tile([128, 1024], F32R, name=f"wp{p}")
            nc.sync.dma_start(w3[:], io["wp"][128 * p:128 * (p + 1), :])
            wp_t.append(w3)
        bqk_t = const.tile([128, 4], F32, name="bqk")
        nc.sync.dma_start(bqk_t[:], io["bqk"][:])
        bv_t = const.tile([128, 256], F32, name="bv")
        nc.sync.dma_start(bv_t[:], io["bv"][:])
        am_t = const.tile([128, NS], F32, name="am")
        nc.sync.dma_start(am_t[:], io["amask"][:])
        cm_t = const.tile([128, 1024], F32R, name="cm")
        nc.sync.dma_start(cm_t[:], io["cmask"][:])
        ones_t = const.tile([128, 64], F32R, name="ones")
        nc.sync.dma_start(ones_t[:], io["ones"][:])

        # ---- qkv outputs ----
        # qk_tiles m-chunks: 0: qT heads {0,1}; 1: qT heads {2,3};
        #                    2: kT heads {0,1}; 3: kT heads {2,3}
        qk_tiles = [qkp.tile([128, T], F32R, name=f"qk{m}") for m in range(4)]
        v_big = [vp.tile([128, 65 * NS], F32R, name=f"vb{h}") for h in range(HG)]
        for h in range(HG):
            # ones column at position 64 of every 65-wide block (softmax
            # denominator accumulator); memset can't write f32r, so copy
            # from the ones tile through a strided AP.
            onescol = v_big[h][:].rearrange("p (s c) -> p s c", c=65)[:, :, 64]
            nc.vector.tensor_copy(onescol, ones_t[:, 0:NS])

        # ---- qkv projection, in two t-halves to bound xT residency ----
        for half in range(2):
            xt = []
            for c in range(KC):
                x1 = xp.tile([128, 1024], F32R, name="xt", tag="xt")
                nc.sync.dma_start(
                    x1[:], io["xT"][128 * c:128 * (c + 1),
                                    1024 * half:1024 * (half + 1)])
                xt.append(x1)
            # qT / kT: weights stationary, xT moving -> transposed outputs
            for m in range(4):
                for tj in range(2):
                    j = 2 * half + tj
                    pq = ps.tile([128, 512], F32, name="pq", tag="ps")
                    for c in range(KC):
                        nc.tensor.matmul(
                            pq[:], wqk_t[c][:, 128 * m:128 * (m + 1)],
                            xt[c][:, 512 * tj:512 * (tj + 1)],
                            start=(c == 0), stop=(c == KC - 1))
                    nc.scalar.activation(
                        qk_tiles[m][:, TCH * j:TCH * (j + 1)], pq[:], IDENT,
                        bias=bqk_t[:, m:m + 1], scale=1.0)
            # v: xT stationary, Wv moving -> natural [s, d] layout
            for si in range(NS // 2):
                s = NS // 2 * half + si
                pv = ps.tile([128, 256], F32, name="pv", tag="ps")
                for c in range(KC):
                    nc.tensor.matmul(
                        pv[:], xt[c][:, 128 * si:128 * (si + 1)], wv_t[c][:],
                        start=(c == 0), stop=(c == KC - 1))
                for h in range(HG):
                    nc.vector.tensor_add(
                        v_big[h][:, 65 * s:65 * s + 64],
                        pv[:, 64 * h:64 * (h + 1)], bv_t[:, 64 * h:64 * (h + 1)])

        # ---- attention ----
        yT = [yp.tile([128, T], F32R, name=f"yT{p}") for p in range(2)]
        # denominator rows: head h -> tile h//2, partition 32*(h%2)
        # (engine APs may only start at partition 0/32/64; 96 is illegal)
        l_t = [yp.tile([64, T], F32, name=f"l{p}") for p in range(2)]
        rl_t = [yp.tile([64, T], F32R, name=f"rl{p}") for p in range(2)]

        # chunk-major so the output projection for chunk j can start as
        # soon as all 4 heads finish chunk j (no serial proj tail).
        for j in range(NJ):
            cols = slice(TCH * j, TCH * (j + 1))
            for h in range(HG):
                pr, hh = divmod(h, 2)
                qt, kt = qk_tiles[pr], qk_tiles[2 + pr]
                rows = slice(64 * hh, 64 * (hh + 1))
                py = ps.tile([128, 512], F32, name="py", tag="ps")
                ns = 4 * (j + 1)
                for i in range(ns):
                    r = i - 4 * j
                    # diagonal blocks r=1,2: the first 128*r t-columns are
                    # entirely below the causal frontier -> skip them.
                    # (r=3 keeps N=512: N=128 would hit the f32r 4x
                    # slow path below N=256.)
                    off = 128 * r if r in (1, 2) else 0
                    sub = slice(off, 512)
                    tsub = slice(TCH * j + off, TCH * (j + 1))
                    pscr = ps.tile([128, 512], F32, name="pscr", tag="ps")
                    nc.tensor.matmul(
                        pscr[:, sub], kt[rows, 128 * i:128 * (i + 1)],
                        qt[rows, tsub], start=True, stop=True)
                    pt = pp.tile([128, 512], F32R, name="pt", tag="pt")
                    nc.scalar.activation(
                        pt[:, sub], pscr[:, sub], EXP,
                        bias=am_t[:, i:i + 1], scale=1.0 / np.sqrt(D))
                    if r >= 0:  # 0/1 triangular mask on the boundary
                        if r == 3:
                            mask = cm_t[:, 512:1024]
                        else:
                            mask = cm_t[:, 0:512 - off]
                        nc.vector.tensor_mul(pt[:, sub], pt[:, sub], mask)
                    nc.tensor.matmul(
                        py[0:65, sub], v_big[h][:, 65 * i:65 * (i + 1)],
                        pt[:, sub], start=(i == 0), stop=(i == ns - 1))
                # drain y rows + denominator row, reciprocal, then
                # broadcast the recip row across 64 partitions via a
                # K=1 ones-row matmul and normalize in place.
                lr = 32 * hh
                nc.vector.tensor_copy(yT[pr][rows, cols], py[0:64, :])
                nc.vector.tensor_copy(l_t[pr][lr:lr + 1, cols],
                                      py[64:65, :])
                nc.vector.reciprocal(rl_t[pr][lr:lr + 1, cols],
                                     l_t[pr][lr:lr + 1, cols])
                pb = ps.tile([128, 512], F32, name="pb", tag="ps")
                nc.tensor.matmul(
                    pb[0:64, :], ones_t[lr:lr + 1, :],
                    rl_t[pr][lr:lr + 1, cols],
                    start=True, stop=True)
                nc.vector.tensor_mul(
                    yT[pr][rows, cols], yT[pr][rows, cols], pb[0:64, :])

            # ---- output projection for chunk j (partial; host sums) ----
            for m in range(8):
                po = ps.tile([128, 512], F32, name="po", tag="ps")
                for pr in range(2):
                    nc.tensor.matmul(
                        po[:], wp_t[pr][:, 128 * m:128 * (m + 1)],
                        yT[pr][:, cols],
                        start=(pr == 0), stop=(pr == 1))
                ot = op.tile([128, 512], F32, name="ot", tag="ot")
                nc.vector.tensor_copy(ot[:], po[:])
                nc.sync.dma_start(
                    io["out"][128 * m:128 * (m + 1), cols], ot[:])


def _build(reps=1):
    nc = bacc.Bacc("TRN2", target_bir_lowering=False, debug=False)
    io = {
        "xT": nc.dram_tensor("xT", [C, T], F32R, kind="ExternalInput").ap(),
        "wqk": nc.dram_tensor("wqk", [C, 512], F32R, kind="ExternalInput").ap(),
        "wv": nc.dram_tensor("wv", [C, 256], F32R, kind="ExternalInput").ap(),
        "wp": nc.dram_tensor("wp", [256, C], F32R, kind="ExternalInput").ap(),
        "bqk": nc.dram_tensor("bqk", [128, 4], F32, kind="ExternalInput").ap(),
        "bv": nc.dram_tensor("bv", [128, 256], F32, kind="ExternalInput").ap(),
        "amask": nc.dram_tensor("amask", [128, NS], F32, kind="ExternalInput").ap(),
        "cmask": nc.dram_tensor("cmask", [128, 1024], F32R, kind="ExternalInput").ap(),
        "ones": nc.dram_tensor("ones", [128, 64], F32R, kind="ExternalInput").ap(),
        "out": nc.dram_tensor("out", [C, T], F32, kind="ExternalOutput").ap(),
    }
    with nc.allow_low_precision("f32r matmul operand staging"):
        with tile.TileContext(nc) as tc:
            _emit(nc, tc, io, reps=reps)
    nc.compile()
    return nc


_NC_CACHE = {}


def _get_nc():
    if "nc" not in _NC_CACHE:
        _NC_CACHE["nc"] = _build()
    return _NC_CACHE["nc"]


def _host_inputs(x, attention_mask, W_attn, b_attn, W_proj):
    """Per-core input dicts implementing the batch x head-group sharding."""
    # causal 0/1 masks: cols 0:512 = (f >= p) for the r=0/1/2 diagonal
    # blocks (truncated per block), cols 512:1024 = (f >= 384 + p) for r=3
    p = np.arange(128)[:, None]
    f = np.arange(512)[None, :]
    cm = np.concatenate(
        [(f >= p).astype(np.float32),
         (f >= 384 + p).astype(np.float32)], axis=1)
    ones = np.ones((128, 64), np.float32)
    in_maps = []
    for c in range(N_CORES):
        b, g = divmod(c, HG)
        q0 = 256 * g
        wqk = np.ascontiguousarray(np.concatenate(
            [W_attn[:, q0:q0 + 256], W_attn[:, C + q0:C + q0 + 256]], axis=1))
        wv = np.ascontiguousarray(W_attn[:, 2 * C + q0:2 * C + q0 + 256])
        wp = np.ascontiguousarray(W_proj[q0:q0 + 256, :])
        bqk = np.stack(
            [b_attn[q0:q0 + 128], b_attn[q0 + 128:q0 + 256],
             b_attn[C + q0:C + q0 + 128], b_attn[C + q0 + 128:C + q0 + 256]],
            axis=1).astype(np.float32)
        bv = np.broadcast_to(
            b_attn[2 * C + q0:2 * C + q0 + 256], (128, 256)).astype(np.float32)
        am = np.ascontiguousarray(
            attention_mask[b, 0, 0].reshape(NS, 128).T.astype(np.float32))
        xT = np.ascontiguousarray(x[b].T)
        in_maps.append(dict(xT=xT, wqk=wqk, wv=wv, wp=wp, bqk=bqk, bv=bv,
                            amask=am, cmask=cm, ones=ones))
    return in_maps


def _assemble(results, b_proj):
    out = np.empty((B, T, C), np.float32)
    for b in range(B):
        acc = np.zeros((C, T), np.float64)
        for g in range(HG):
            acc += results[HG * b + g]["out"].astype(np.float64)
        out[b] = acc.T + b_proj[None, :]
    return out


def kernel(x, attention_mask, W_attn, b_attn, W_proj, b_proj):
    x = np.asarray(x, np.float32)
    attention_mask = np.asarray(attention_mask, np.float32)
    W_attn = np.asarray(W_attn, np.float32)
    b_attn = np.asarray(b_attn, np.float32)
    W_proj = np.asarray(W_proj, np.float32)
    b_proj = np.asarray(b_proj, np.float32)

    nc = _get_nc()
    in_maps = _host_inputs(x, attention_mask, W_attn, b_attn, W_proj)
    res = run_bass_kernel_spmd(nc, in_maps, list(range(N_CORES)))
    return _assemble(res.results, b_proj)


# revision 46
# speedup vs baseline: 2.0863x; 1.4188x over previous
"""Causal self-attention (B=2, T=2048, C=1024, H=16) on 8 trn2 NeuronCores.

Sharding: core c -> batch b=c//4 (data parallel) x head-group g=c%4
(tensor parallel, 4 heads each). Each core computes qkv projections for
its 4 heads, causal flash-style attention, and a partial output
projection (its heads' rows of W_proj); the host sums the 4 partials
per batch and adds b_proj.

Device layout avoids all on-chip transposes:
  - q,k are produced transposed ([head_dim*heads, T], dims on partitions)
    straight out of the qkv matmul (W as stationary, xT as moving).
  - v is produced in natural [T, head_dim] layout (xT as stationary),
    augmented with a ones column so P@v_aug also yields the softmax
    denominator.
  - scores are computed as S^T = k @ q^T in [s, t] layout, exp'ed with
    no max subtraction (scores are O(1); fully-masked blocks are
    skipped, diagonal blocks multiplied by a 0/1 triangular mask, which
    matches exp(-10000+...) == 0.0 in fp32 exactly).
  - y^T = v_aug^T @ P accumulates in PSUM; the denominator row is
    reciprocal'ed and broadcast across partitions via a tiny selector
    matmul, then multiplied in.
  - the output projection consumes y^T directly as the moving operand
    (W_proj slice stationary), producing the partial output transposed.

All matmul operands are float32r (1 cycle/row at N>=512 vs 4 for fp32;
measured dot-product rel-err ~1.5e-4 at K=128).
"""

import numpy as np

import concourse.bass as bass
import concourse.tile as tile
from concourse import bacc, mybir
from concourse.bass_utils import run_bass_kernel_spmd

F32 = mybir.dt.float32
F32R = mybir.dt.float32r
EXP = mybir.ActivationFunctionType.Exp
IDENT = mybir.ActivationFunctionType.Identity

B, T, C, H = 2, 2048, 1024, 16
D = C // H                    # 64
N_CORES = 8
HG = 4                        # heads per core
TCH = 512                     # t-chunk (moving free dim)
NJ = T // TCH                 # 4 t-chunks
NS = T // 128                 # 16 s-tiles
KC = C // 128                 # 8 contraction chunks


def _emit(nc, tc, io, reps=1, fast=True):
    import contextlib
    ctx = contextlib.ExitStack()
    with ctx:
        if reps > 1:
            # timing builds only: repeat the whole body on-device so the
            # per-iteration time can be measured free of host dispatch
            # noise ((t(reps) - t(1)) / (reps - 1)).
            hints = (mybir.EngineType.PE, mybir.EngineType.SP,
                     mybir.EngineType.Activation, mybir.EngineType.DVE)
            ctx.enter_context(tc.For_i(0, reps, 1, hint_engines=hints))
        const = ctx.enter_context(tc.tile_pool(name="const", bufs=1))
        xp = ctx.enter_context(tc.tile_pool(name="xp", bufs=10))
        qkp = ctx.enter_context(tc.tile_pool(name="qkp", bufs=1))
        vp = ctx.enter_context(tc.tile_pool(name="vp", bufs=1))
        pp = ctx.enter_context(tc.tile_pool(name="pp", bufs=5))
        yp = ctx.enter_context(tc.tile_pool(name="yp", bufs=1))
        op = ctx.enter_context(tc.tile_pool(name="op", bufs=3))
        # PSUM: three dedicated pools so score-psum slots are only ever
        # reused for scores (bounded stale values -> batched exp is safe)
        # and so py/pb/po never contend with the score double-buffer.
        # 2*2 + 2*1 + 2*1 = 8 banks.
        scp = ctx.enter_context(tc.tile_pool(name="scp", bufs=2, space="PSUM"))
        pyp = ctx.enter_context(tc.tile_pool(name="pyp", bufs=2, space="PSUM"))
        pbo = ctx.enter_context(tc.tile_pool(name="pbo", bufs=2, space="PSUM"))

        # exp-table prefetch: a dummy activation with no input deps, so the
        # ~2.7us ACT table load overlaps the initial DMA ramp.
        scratch = const.tile([128, 1], F32, name="scratch")
        nc.vector.memset(scratch[:], 0.0)
        nc.scalar.activation(scratch[:], scratch[:], EXP)

        # ---- weights, first-needed first (wqk/xT feed the first matmul) ----
        wqk_t = []
        xt_half = {}
        for c in range(KC):
            w1 = const.tile([128, 512], F32R, name=f"wqk{c}")
            nc.sync.dma_start(w1[:], io["wqk"][128 * c:128 * (c + 1), :])
            wqk_t.append(w1)
            x1 = xp.tile([128, 1024], F32R, name="xt", tag="xt")
            nc.sync.dma_start(x1[:], io["xT"][128 * c:128 * (c + 1), 0:1024])
            xt_half.setdefault(0, []).append(x1)
        wv_t = []
        for c in range(KC):
            w2 = const.tile([128, 256], F32R, name=f"wv{c}")
            nc.sync.dma_start(w2[:], io["wv"][128 * c:128 * (c + 1), :])
            wv_t.append(w2)
        bqk_t = const.tile([128, 4], F32, name="bqk")
        nc.sync.dma_start(bqk_t[:], io["bqk"][:])
        bv_t = const.tile([128, 256], F32, name="bv")
        nc.sync.dma_start(bv_t[:], io["bv"][:])
        am_t = const.tile([128, NS], F32, name="am")
        nc.sync.dma_start(am_t[:], io["amask"][:])
        cm_t = const.tile([128, 2048], F32R, name="cm")
        nc.sync.dma_start(cm_t[:], io["cmask"][:])
        ones_t = const.tile([128, 64], F32R, name="ones")
        nc.sync.dma_start(ones_t[:], io["ones"][:])
        wp_t = []
        for p in range(2):
            w3 = const.tile([128, 1024], F32R, name=f"wp{p}")
            nc.sync.dma_start(w3[:], io["wp"][128 * p:128 * (p + 1), :])
            wp_t.append(w3)

        # ---- qkv outputs ----
        # qk_tiles m-chunks: 0: qT heads {0,1}; 1: qT heads {2,3};
        #                    2: kT heads {0,1}; 3: kT heads {2,3}
        qk_tiles = [qkp.tile([128, T], F32R, name=f"qk{m}") for m in range(4)]
        # all 4 heads' v_aug side by side: head h block at 1040*h, s-tile i
        # at 1040*h + 65*i, with a ones column at offset 64 of each block
        # (softmax denominator accumulator).
        v_all = vp.tile([128, 4 * 65 * NS], F32R, name="v_all")
        onescol = v_all[:].rearrange("p (h s c) -> p h s c", h=HG, c=65)[:, :, :, 64]
        nc.vector.tensor_copy(
            onescol, ones_t[:, 0:HG * NS].rearrange("p (h s) -> p h s", h=HG))

        # ---- qkv projection, in two t-halves to bound xT residency ----
        def qkv_half(half):
            if half == 1:
                xs = []
                for c in range(KC):
                    x1 = xp.tile([128, 1024], F32R, name="xt", tag="xt")
                    nc.sync.dma_start(
                        x1[:], io["xT"][128 * c:128 * (c + 1), 1024:2048])
                    xs.append(x1)
                xt_half[1] = xs
            xt = xt_half[half]
            # qT / kT: weights stationary, xT moving -> transposed outputs.
            # c-chunk outer with 4 concurrent accumulator groups so the PE
            # gets 4 matmuls of work per arriving xT chunk during the
            # initial DMA ramp (m=0,1 in scp slots, m=2,3 in pbo slots).
            for tj in range(2):
                j = 2 * half + tj
                pqs = [scp.tile([128, 512], F32, name=f"pqa{m}", tag="scp")
                       for m in range(2)]
                pqs += [pbo.tile([128, 512], F32, name=f"pqb{m}", tag="pbo")
                        for m in range(2)]
                for c in range(KC):
                    for m in range(4):
                        nc.tensor.matmul(
                            pqs[m][:], wqk_t[c][:, 128 * m:128 * (m + 1)],
                            xt[c][:, 512 * tj:512 * (tj + 1)],
                            start=(c == 0), stop=(c == KC - 1))
                for m in range(4):
                    dst = qk_tiles[m][:, TCH * j:TCH * (j + 1)]
                    if fast:  # b_attn == 0: plain drain on DVE
                        nc.vector.tensor_copy(dst, pqs[m][:])
                    else:
                        nc.scalar.activation(dst, pqs[m][:], IDENT,
                                             bias=bqk_t[:, m:m + 1], scale=1.0)
            # v: xT stationary, Wv moving -> natural [s, d] layout
            for si in range(NS // 2):
                s = NS // 2 * half + si
                pv = pyp.tile([128, 256], F32, name="pv", tag="pyp")
                for c in range(KC):
                    nc.tensor.matmul(
                        pv[:], xt[c][:, 128 * si:128 * (si + 1)], wv_t[c][:],
                        start=(c == 0), stop=(c == KC - 1))
                # one strided drain for all 4 heads' slices of this s-tile
                vdst = v_all[:].rearrange(
                    "p (h s c) -> p h s c", h=HG, c=65)[:, :, s, 0:64]
                nc.vector.tensor_add(
                    vdst, pv[:].rearrange("p (h d) -> p h d", h=HG),
                    bv_t[:].rearrange("p (h d) -> p h d", h=HG))

        # ---- attention ----
        yT = [yp.tile([128, T], F32R, name=f"yT{p}") for p in range(2)]
        # denominator recip rows: head h -> tile h//2, partition 32*(h%2)
        # (engine APs may only start at partition 0/32/64; 96 is illegal)
        rl_t = [yp.tile([64, T], F32R, name=f"rl{p}") for p in range(2)]

        def _bcast_norm(h, j, py):
            # broadcast head h's recip row across 64 partitions via a K=1
            # ones-row matmul, then normalize yT in place (an engine op may
            # read at most ONE operand from PSUM, so py was drained to yT
            # by a DVE copy first).
            pr, hh = divmod(h, 2)
            rows = slice(64 * hh, 64 * (hh + 1))
            lr = 32 * hh
            cols = slice(TCH * j, TCH * (j + 1))
            pb = pbo.tile([128, 512], F32, name="pb", tag="pbo")
            nc.tensor.matmul(
                pb[0:64, :], ones_t[lr:lr + 1, :],
                rl_t[pr][lr:lr + 1, cols], start=True, stop=True)
            nc.vector.tensor_mul(yT[pr][rows, cols], yT[pr][rows, cols],
                                 pb[0:64, :])

        # attention for one t-chunk; bcast+normalize deferred one head so
        # the PE never head-of-line blocks on the DVE recip chain
        def attn_chunk(j):
            pending = None
            cols = slice(TCH * j, TCH * (j + 1))
            for h in range(HG):
                pr, hh = divmod(h, 2)
                qt, kt = qk_tiles[pr], qk_tiles[2 + pr]
                rows = slice(64 * hh, 64 * (hh + 1))
                py = pyp.tile([128, 512], F32, name="py", tag="pyp")
                ns = 4 * (j + 1)
                # s-tiles in pairs: scores into a 2-bank psum tile, one
                # batched exp over both, one combined causal mask for the
                # two diagonal pairs. The PE queue is in-order, so each
                # pair's PV matmuls are deferred until after the NEXT
                # pair's scores (the exp runs in between), and the two
                # diagonal pairs -- whose mask adds gpsimd latency -- are
                # scored first but PV'd last.
                gseq = [2 * j, 2 * j + 1] + list(range(2 * j))
                pvseq = list(range(2 * j)) + [2 * j, 2 * j + 1]
                pts = {}

                def _pv(g, start, stop):
                    for b in range(2):
                        i = 2 * g + b
                        nc.tensor.matmul(
                            py[0:65, :],
                            v_all[:, 1040 * h + 65 * i:1040 * h + 65 * (i + 1)],
                            pts[g][:, 512 * b:512 * (b + 1)],
                            start=(start and b == 0), stop=(stop and b == 1))

                npv = 0
                for gn, g in enumerate(gseq):
                    pscr = scp.tile([128, 1024], F32, name="pscr", tag="scp")
                    for b in range(2):
                        i = 2 * g + b
                        nc.tensor.matmul(
                            pscr[:, 512 * b:512 * (b + 1)],
                            kt[rows, 128 * i:128 * (i + 1)],
                            qt[rows, cols], start=True, stop=True)
                    pt = pp.tile([128, 1024], F32R, name="pt", tag="pt")
                    pts[g] = pt
                    if fast:
                        # attention_mask is all-zero: no per-s bias needed
                        nc.scalar.activation(
                            pt[:], pscr[:], EXP, scale=1.0 / np.sqrt(D))
                    else:
                        for b in range(2):
                            i = 2 * g + b
                            nc.scalar.activation(
                                pt[:, 512 * b:512 * (b + 1)],
                                pscr[:, 512 * b:512 * (b + 1)], EXP,
                                bias=am_t[:, i:i + 1], scale=1.0 / np.sqrt(D))
                    d = g - 2 * j
                    if d >= 0:  # diagonal pair: combined 0/1 causal mask.
                        eng = nc.gpsimd if j > 0 else nc.vector
                        eng.tensor_mul(
                            pt[:], pt[:], cm_t[:, 1024 * d:1024 * (d + 1)])
                    # one deferred PV per emitted score-pair, starting once
                    # the first full pair's scores are in flight
                    if gn >= 2 and npv < len(pvseq):
                        _pv(pvseq[npv], npv == 0, npv == len(pvseq) - 1)
                        npv += 1
                while npv < len(pvseq):
                    _pv(pvseq[npv], npv == 0, npv == len(pvseq) - 1)
                    npv += 1
                # drain y rows; reciprocal of the denominator row straight
                # from PSUM
                lr = 32 * hh
                nc.vector.tensor_copy(yT[pr][rows, cols], py[0:64, :])
                nc.vector.reciprocal(rl_t[pr][lr:lr + 1, cols], py[64:65, :])
                if pending is not None:
                    _bcast_norm(*pending)
                pending = (h, j, py)
            _bcast_norm(*pending)

        # ---- output projection for chunk j (partial; host sums) ----
        def proj_chunk(j):
            cols = slice(TCH * j, TCH * (j + 1))
            for m in range(8):
                po = pbo.tile([128, 512], F32, name="po", tag="pbo")
                for pr in range(2):
                    nc.tensor.matmul(
                        po[:], wp_t[pr][:, 128 * m:128 * (m + 1)],
                        yT[pr][:, cols],
                        start=(pr == 0), stop=(pr == 1))
                ot = op.tile([128, 512], F32, name="ot", tag="ot")
                nc.vector.tensor_copy(ot[:], po[:])
                nc.sync.dma_start(
                    io["out"][128 * m:128 * (m + 1), cols], ot[:])

        # Interleave phases: attention is ACT(exp)-bound while qkv/proj are
        # PE-bound, and engine queues execute in issue order -- so emit
        # attention chunks 0-1 (which only need half-0 qkv outputs) before
        # qkv half 1 to keep both engines fed.
        qkv_half(0)
        attn_chunk(0)
        attn_chunk(1)
        qkv_half(1)
        proj_chunk(0)
        proj_chunk(1)
        attn_chunk(2)
        proj_chunk(2)
        attn_chunk(3)
        proj_chunk(3)


def _build(reps=1, fast=True):
    nc = bacc.Bacc("TRN2", target_bir_lowering=False, debug=False)
    io = {
        "xT": nc.dram_tensor("xT", [C, T], F32R, kind="ExternalInput").ap(),
        "wqk": nc.dram_tensor("wqk", [C, 512], F32R, kind="ExternalInput").ap(),
        "wv": nc.dram_tensor("wv", [C, 256], F32R, kind="ExternalInput").ap(),
        "wp": nc.dram_tensor("wp", [256, C], F32R, kind="ExternalInput").ap(),
        "bqk": nc.dram_tensor("bqk", [128, 4], F32, kind="ExternalInput").ap(),
        "bv": nc.dram_tensor("bv", [128, 256], F32, kind="ExternalInput").ap(),
        "amask": nc.dram_tensor("amask", [128, NS], F32, kind="ExternalInput").ap(),
        "cmask": nc.dram_tensor("cmask", [128, 2048], F32R, kind="ExternalInput").ap(),
        "ones": nc.dram_tensor("ones", [128, 64], F32R, kind="ExternalInput").ap(),
        "out": nc.dram_tensor("out", [C, T], F32, kind="ExternalOutput").ap(),
    }
    with nc.allow_low_precision("f32r matmul operand staging"):
        with tile.TileContext(nc) as tc:
            _emit(nc, tc, io, reps=reps, fast=fast)
    nc.compile()
    return nc


_NC_CACHE = {}


def _get_nc(fast=True):
    key = ("nc", fast)
    if key not in _NC_CACHE:
        _NC_CACHE[key] = _build(fast=fast)
    return _NC_CACHE[key]


def _host_inputs(x, attention_mask, W_attn, b_attn, W_proj):
    """Per-core input dicts implementing the batch x head-group sharding."""
    # causal 0/1 masks for the 4 diagonal-block offsets, as one [128, 2048]
    p = np.arange(128)[:, None]
    f = np.arange(512)[None, :]
    cm = np.concatenate(
        [(f >= 128 * r + p).astype(np.float32) for r in range(4)], axis=1)
    ones = np.ones((128, 64), np.float32)
    in_maps = []
    for c in range(N_CORES):
        b, g = divmod(c, HG)
        q0 = 256 * g
        wqk = np.ascontiguousarray(np.concatenate(
            [W_attn[:, q0:q0 + 256], W_attn[:, C + q0:C + q0 + 256]], axis=1))
        wv = np.ascontiguousarray(W_attn[:, 2 * C + q0:2 * C + q0 + 256])
        wp = np.ascontiguousarray(W_proj[q0:q0 + 256, :])
        bqk = np.stack(
            [b_attn[q0:q0 + 128], b_attn[q0 + 128:q0 + 256],
             b_attn[C + q0:C + q0 + 128], b_attn[C + q0 + 128:C + q0 + 256]],
            axis=1).astype(np.float32)
        bv = np.broadcast_to(
            b_attn[2 * C + q0:2 * C + q0 + 256], (128, 256)).astype(np.float32)
        am = np.ascontiguousarray(
            attention_mask[b, 0, 0].reshape(NS, 128).T.astype(np.float32))
        xT = np.ascontiguousarray(x[b].T)
        in_maps.append(dict(xT=xT, wqk=wqk, wv=wv, wp=wp, bqk=bqk, bv=bv,
                            amask=am, cmask=cm, ones=ones))
    return in_maps


def _assemble(results, b_proj):
    out = np.empty((B, T, C), np.float32)
    for b in range(B):
        acc = np.zeros((C, T), np.float64)
        for g in range(HG):
            acc += results[HG * b + g]["out"].astype(np.float64)
        out[b] = acc.T + b_proj[None, :]
    return out


def kernel(x, attention_mask, W_attn, b_attn, W_proj, b_proj):
    x = np.asarray(x, np.float32)
    attention_mask = np.asarray(attention_mask, np.float32)
    W_attn = np.asarray(W_attn, np.float32)
    b_attn = np.asarray(b_attn, np.float32)
    W_proj = np.asarray(W_proj, np.float32)
    b_proj = np.asarray(b_proj, np.float32)

    # fast build: zero attention_mask (batched exp, no per-s bias) and
    # zero qkv bias (plain DVE drains); anything else gets the general
    # per-block-bias variant
    fast = not (np.any(attention_mask) or np.any(b_attn))
    nc = _get_nc(fast=fast)
    in_maps = _host_inputs(x, attention_mask, W_attn, b_attn, W_proj)
    res = run_bass_kernel_spmd(nc, in_maps, list(range(N_CORES)))
    return _assemble(res.results, b_proj)
